# revision 44
# baseline (speedup 1.0000x reference)
"""Fused causal-transformer block (LN1 -> attn -> LN2 -> MLP, residuals) on
8 Trainium2 NeuronCores — fp8 DoubleRow edition.

Sharding: tensor-parallel attention (4 heads/core) x data-parallel batch,
with a SEQUENCE-PARALLEL MLP: the attention out-projection partials are
ReduceScattered over each 4-core group, so core s owns four 128-token
blocks (one per 512-token chunk) and runs the whole MLP (full 4096 hidden)
on just those tokens.  No second collective is needed — each core writes
its own final output rows.

Compute dtype: fp8e4 (e4m3) matmul operands with DoubleRow perf mode
(2 k-tiles per instruction, 0.5 cycles/row), fp32 PSUM accumulation,
bf16 LayerNorm stream, fp32 residual stream, bf16 collective wire.

Layouts (per core):
  h1T        [128, C/128, T]   LN1 output transposed, fp8
  q_sb/k_sb  [128, 2, T]       per head h: partitions 32h..32h+31 hold the
                               two 32-dim halves of q_h (resp k_h) in free
                               slots 0/1 -> scores run as DoubleRow with
                               K=32x2 (full 64-dim dot at 0.5 cyc/row).
  vaug       [128, T/128, 4, 64] v token-major per head, fp8
  yT         [128, 2, T]       attention output transposed, fp8
  psy tile   [65, 512] PSUM    rows 0..63 = sum(p*v), row 64 = sum(p)
                               (via a ones[128,2,1] DoubleRow matmul)
"""

import contextlib
import ctypes
import math
import sys
import types

import numpy as np
import ml_dtypes

import bass_rust
import concourse.bass as bass
import concourse.mybir as mybir
import concourse.tile as tile
from concourse import library_config
from concourse.bass_utils import run_bass_kernel_spmd
from concourse.masks import make_identity
from concourse.tile import TileContext
from concourse.vector_clock import ScopedClock

# ---------------------------------------------------------------------------
# problem constants (hardcoded per the harness contract)
B, T, C, H = 2, 2048, 1024, 16
HD = C // H                 # 64
N_CORES = 8
TPG = 4                     # tensor-parallel group size
H_CORE = H // TPG           # heads per core = 4
DH = H_CORE * HD            # per-core attention dim = 256
FH = 4 * C                  # full MLP hidden (sequence-parallel MLP)
P = 128
EPS = 1e-5
QCH = 512                   # q-chunk width
NQC = T // QCH              # 4 q chunks
TSL = T // TPG              # per-core MLP token count = 512
GROUPS = [[0, 1, 2, 3], [4, 5, 6, 7]]

F32 = mybir.dt.float32
BF16 = mybir.dt.bfloat16
FP8 = mybir.dt.float8e4
NP_FP8 = ml_dtypes.float8_e4m3
DR = mybir.MatmulPerfMode.DoubleRow

# fp8 scaling: weights are quantized at x32 (fits e4m3 normal range);
# chunk-0 attention (tokens 0..511) runs in bf16 because its nearly-
# diagonal softmax gives no error averaging.  The MLP runs in bf16.
WS = 1.0                   # fp8 weight pre-scale (w_qk, w_v, w_ap)
YS = 1.0                   # y quantization pre-scale
APS = WS * YS              # aproj partial scale on the RS wire
SCORE_SCALE = 1.0          # exp scale for fp8 chunks (q pre-scaled x0.125)
SCORE_SCALE0 = 0.125       # exp scale for the bf16 chunk

# ---------------------------------------------------------------------------
# workaround 1: the container's walrus accepts a single sync-wait command per
# instruction; move extra semaphore waits onto inserted EventSemaphore
# instructions on the same engine (program order preserves semantics).

_waitfix_counter = [0]


def _legalize_waits(nc, cap=1):
    fn = nc.m.functions[0]
    n_split = 0
    for bb in fn.blocks:
        out = []
        changed = False
        for inst in bb.instructions:
            si = inst.sync_info
            waits = list(si.on_wait) if si is not None else []
            if len(waits) > cap:
                movable = [w for w in waits if w.sync_type == "semaphore"]
                fixed = [w for w in waits if w.sync_type != "semaphore"]
                n_keep = max(cap - len(fixed), 0)
                keep = fixed + (movable[len(movable) - n_keep:] if n_keep else [])
                extra = movable[: len(movable) - n_keep] if n_keep else movable
                for w in extra:
                    _waitfix_counter[0] += 1
                    ev = mybir.InstEventSemaphore(
                        name=f"I-waitfix-{_waitfix_counter[0]}",
                        engine=inst.engine,
                        ins=[],
                        outs=[],
                        sync_info=bass_rust.SyncInfo(on_wait=[w], on_update=[]),
                    )
                    out.append(ev)
                    n_split += 1
                inst.sync_info = bass_rust.SyncInfo(
                    on_wait=keep, on_update=list(si.on_update)
                )
                changed = True
            out.append(inst)
        if changed:
            bb.instructions = out
    return n_split


# workaround 2: same issue for the Tile kernel-tail Drain — emit one wait-nop
# per live proc ahead of a wait-less drain instead of stacking waits on it.


def _drain_and_barrier_split(self, tick_clock, wait_clock):
    gc = tick_clock.global_clock
    sems_alloc = wait_clock.sems.allocated()
    for proc in sorted(sems_alloc):
        tick = gc.peek_next(proc) - 1
        if tick <= 0:
            continue
        vc1 = bass_rust.VectorClock()
        vc1.require_at_least(proc, tick)
        nop = self.nc.sync.nop()
        wait_clock.add_sem_waits(nop.ins, ScopedClock({None: vc1}))
    self.nc.sync.drain()
    self.nc.all_engine_barrier()
    assert self.sems is not None
    popped = self.nc._tile_sem_poison_stack.pop()
    assert popped is self._sem_poison
    self.nc.clear_and_free_semaphores(list(self.sems.allocated().values()))
    self.nc.all_engine_barrier()


TileContext._drain_and_barrier = _drain_and_barrier_split


# workaround 3 (profiling only): register the NTFF hook the trimmed antenv
# lacks so run_bass_kernel_spmd(trace=True) works under axon.


def _install_prof_hook():
    if "antenv.axon_hooks" in sys.modules:
        return
    so_path = "/opt/axon/libaxon_pjrt.so"
    hook = None
    try:
        lib = ctypes.CDLL(so_path)
        if hasattr(lib, "axon_start_nrt_profile"):
            lib.axon_start_nrt_profile.argtypes = [
                ctypes.POINTER(ctypes.c_int64),
                ctypes.c_size_t,
            ]
            lib.axon_start_nrt_profile.restype = ctypes.c_int64
            lib.axon_stop_nrt_profile.argtypes = [ctypes.c_char_p]
            lib.axon_stop_nrt_profile.restype = ctypes.c_int64

            @contextlib.contextmanager
            def _hook_cm(output_dir, device_ids):
                import jax

                jax.devices()
                if device_ids:
                    ids = (ctypes.c_int64 * len(device_ids))(*device_ids)
                    rc = lib.axon_start_nrt_profile(ids, len(device_ids))
                else:
                    rc = lib.axon_start_nrt_profile(None, 0)
                if rc != 0:
                    raise RuntimeError(f"axon_start_nrt_profile rc={rc}")
                try:
                    yield
                finally:
                    n = lib.axon_stop_nrt_profile(str(output_dir).encode())
                    if n < 0:
                        raise RuntimeError(f"axon_stop_nrt_profile rc={n}")

            hook = _hook_cm
    except OSError:
        pass
    mod = types.ModuleType("antenv.axon_hooks")
    mod.get_axon_ntff_profile_hook = lambda: hook
    mod.set_axon_ntff_profile_hook = lambda h: None
    sys.modules["antenv.axon_hooks"] = mod
    from concourse import bass_utils

    bass_utils.upload_artifacts = lambda tmpdir: tmpdir


# ---------------------------------------------------------------------------
# device kernel builder


def build_module(
    flags=frozenset(),
    replica_groups=GROUPS,
    local_reduce=False,
    legalize=True,
):
    """Build the per-core SPMD Bass module."""
    KO = C // P                 # 8 c-tiles
    NT = T // P                 # 16 token tiles
    KPQ = QCH // P              # 4 token tiles per chunk
    FKO = FH // P               # 32 hidden tiles
    NCC = C // QCH              # 2

    nc = bass.Bass(num_devices=N_CORES)

    tri_in = nc.dram_tensor("tri", (P, P), BF16, kind="ExternalInput")
    x_bf = nc.dram_tensor("x_bf", (T, C), BF16, kind="ExternalInput")
    x_sl = nc.dram_tensor("x_sl", (TSL, C), F32, kind="ExternalInput")
    w_qk = nc.dram_tensor("w_qk", (P, KO, 4 * P), FP8, kind="ExternalInput")
    w_qk0 = nc.dram_tensor("w_qk0", (P, KO, 4 * P), BF16, kind="ExternalInput")
    w_v = nc.dram_tensor("w_v", (P, KO, DH), FP8, kind="ExternalInput")
    w_v0 = nc.dram_tensor("w_v0", (P, KO, DH), BF16, kind="ExternalInput")
    w_ap = nc.dram_tensor("w_ap", (P, 2, C), FP8, kind="ExternalInput")
    w_ap0 = nc.dram_tensor("w_ap0", (P, 2, C), BF16, kind="ExternalInput")
    w_fc = nc.dram_tensor("w_fc", (P, KO, FH), BF16, kind="ExternalInput")
    w_mp = nc.dram_tensor("w_mp", (P, FKO, C), BF16, kind="ExternalInput")
    opt_in = {}
    for name, shape in [
        ("ln1_g", (1, C)), ("ln1_b", (1, C)),
        ("ln2_g", (1, C)), ("ln2_b", (1, C)),
        ("b_qk", (64, 8)), ("b_qk0", (P, 4)), ("b_v", (1, DH)),
        ("b_ap", (1, C)), ("b_fc", (1, FH)), ("b_mp", (1, C)),
    ]:
        if name in flags or (name == "b_qk0" and "b_qk" in flags):
            opt_in[name] = nc.dram_tensor(name, shape, F32, kind="ExternalInput")

    out_y = nc.dram_tensor("out", (TSL, C), F32, kind="ExternalOutput")

    ARDT = BF16
    rs_in = [nc.dram_tensor(f"rs_in{i}", (QCH, C), ARDT) for i in range(NQC)]
    rs_out = [nc.dram_tensor(f"rs_out{i}", (P, C), ARDT) for i in range(NQC)]

    with TileContext(nc) as tc, contextlib.ExitStack() as ctx:
        const = ctx.enter_context(tc.tile_pool(name="const", bufs=1))
        workb = ctx.enter_context(tc.tile_pool(name="workb", bufs=2))
        works = ctx.enter_context(tc.tile_pool(name="works", bufs=2))
        stats = ctx.enter_context(tc.tile_pool(name="stats", bufs=6))
        rowp = ctx.enter_context(tc.tile_pool(name="rows", bufs=4))

        ident_bf = const.tile([P, P], BF16)
        make_identity(nc, ident_bf)
        ident_f8 = const.tile([P, P], FP8)
        nc.vector.tensor_copy(ident_f8[:], ident_bf[:])
        ones_c0_f = const.tile([1, 64], F32, name="ones_c0")
        nc.vector.memset(ones_c0_f[:], 1.0)
        ones_c0 = ones_c0_f[:].bitcast(mybir.dt.float32r)
        ones_sc_f = const.tile([1, 64], F32, name="ones_sc")
        nc.vector.memset(ones_sc_f[:], YS / WS)
        ones_sc = ones_sc_f[:].bitcast(mybir.dt.float32r)
        eps_t = const.tile([P, 1], F32)
        nc.vector.memset(eps_t[:], EPS)
        tri_sb = const.tile([P, P], BF16)
        nc.sync.dma_start(tri_sb[:], tri_in[:])

        def _bcast_row(name, width):
            if name not in opt_in:
                return None
            bc = const.tile([P, width], F32, name=f"bc_{name}", tag=f"bc_{name}")
            nc.sync.dma_start(bc[:], opt_in[name][:].to_broadcast((P, width)))
            return bc

        def _row(name):
            if name not in opt_in:
                return None
            t_ = const.tile(list(opt_in[name].shape), F32, name=f"r_{name}",
                            tag=f"r_{name}")
            nc.sync.dma_start(t_[:], opt_in[name][:])
            return t_

        ln1_g_bc = _bcast_row("ln1_g", C)
        ln1_b_bc = _bcast_row("ln1_b", C)
        ln2_g_bc = _bcast_row("ln2_g", C)
        ln2_b_bc = _bcast_row("ln2_b", C)
        b_v_bc = _bcast_row("b_v", DH)
        b_ap_bc = _bcast_row("b_ap", C)
        b_qk_col = _row("b_qk")      # [64, 8] for fp8 chunks
        b_qk0_col = _row("b_qk0")    # [P, 4] for the bf16 chunk
        b_fc_row = _row("b_fc")      # [1, FH]
        b_mp_row = _row("b_mp")      # [1, C]
        ones_tok = None
        if b_fc_row is not None or b_mp_row is not None:
            ones_tok = const.tile([1, P], BF16, name="ones_tok", tag="ones_tok")
            nc.vector.memset(ones_tok[:], 1.0)

        # w_fc is persistent (prefetched at kernel start; used in MLP phase)
        wfcp = ctx.enter_context(tc.tile_pool(name="wfcp", bufs=1))
        w_fc_sb = wfcp.tile([P, KO, FH], BF16)
        nc.sync.dma_start(w_fc_sb[:], w_fc[:])

        def ln_tile(x_in, g_bc, b_bc, width=C):
            """LayerNorm of a [P, width] tile -> new [P, width] bf16 tile."""
            nsub = width // 512
            st = stats.tile([P, nsub, 6], F32)
            for j in range(nsub):
                nc.vector.bn_stats(st[:, j, :], x_in[:, j * 512:(j + 1) * 512])
            mv = stats.tile([P, 2], F32)
            nc.vector.bn_aggr(mv[:], st[:])
            r = stats.tile([P, 1], F32)
            nc.scalar.activation(
                r[:], mv[:, 1:2], mybir.ActivationFunctionType.Sqrt, bias=eps_t[:]
            )
            nc.vector.reciprocal(r[:], r[:])
            h_bf = works.tile([P, width], BF16, tag="ln_out")
            if g_bc is None and b_bc is None:
                nc.vector.tensor_scalar(
                    out=h_bf[:], in0=x_in[:], scalar1=mv[:, 0:1], scalar2=r[:],
                    op0=mybir.AluOpType.subtract, op1=mybir.AluOpType.mult,
                )
            else:
                h_f = workb.tile([P, width], F32, tag="ln_f32")
                nc.vector.tensor_scalar(
                    out=h_f[:], in0=x_in[:], scalar1=mv[:, 0:1], scalar2=r[:],
                    op0=mybir.AluOpType.subtract, op1=mybir.AluOpType.mult,
                )
                if g_bc is not None:
                    nc.vector.tensor_mul(h_f[:], h_f[:], g_bc[:])
                if b_bc is not None:
                    nc.vector.tensor_add(h_f[:], h_f[:], b_bc[:])
                nc.vector.tensor_copy(h_bf[:], h_f[:])
            return h_bf

        def transpose_bf(dstT, src_bf, tt, ps_tr, nko=KO):
            """PE-transpose [P, nko*128] bf16 -> bf16 dstT[:, :, tt*P:..]."""
            for kg in range(0, nko, 4):
                nb = min(4, nko - kg)
                pst = ps_tr.tile([P, 4 * P], BF16, tag="pstb")
                for j in range(nb):
                    nc.tensor.transpose(
                        pst[:, j * P:(j + 1) * P],
                        src_bf[:, (kg + j) * P:(kg + j + 1) * P],
                        ident_bf[:],
                    )
                nc.vector.tensor_copy(
                    dstT[:, kg:kg + nb, tt * P:(tt + 1) * P],
                    pst[:, 0:nb * P].rearrange("p (a b) -> p a b", a=nb),
                )

        def transpose_f8(dstT, src_f8, tt, ps_tr, nko=KO):
            """PE-transpose [P, nko*128] fp8 -> fp8 dstT (stride-2 PSUM)."""
            for kg in range(0, nko, 4):
                nb = min(4, nko - kg)
                pst = ps_tr.tile([P, 4 * P, 2], FP8, tag="pst8")
                for j in range(nb):
                    nc.tensor.transpose(
                        pst[:, j * P:(j + 1) * P, 0:1],
                        src_f8[:, (kg + j) * P:(kg + j + 1) * P],
                        ident_f8[:],
                    )
                nc.vector.tensor_copy(
                    dstT[:, kg:kg + nb, tt * P:(tt + 1) * P],
                    pst[:, 0:nb * P, 0:1].rearrange(
                        "p (a b) o -> p a (b o)", a=nb
                    ),
                )

        # ============================ attention ============================
        with contextlib.ExitStack() as attn_ctx:
            wa = attn_ctx.enter_context(tc.tile_pool(name="wa", bufs=1))
            w_qk0_sb = wa.tile([P, KO, 4 * P], BF16)
            nc.sync.dma_start(w_qk0_sb[:], w_qk0[:])
            w_v0_sb = wa.tile([P, KO, DH], BF16)
            nc.sync.dma_start(w_v0_sb[:], w_v0[:])
            w_ap0_sb = wa.tile([P, 2, C], BF16)
            nc.sync.dma_start(w_ap0_sb[:], w_ap0[:])

            big = attn_ctx.enter_context(tc.tile_pool(name="attn_big", bufs=1))
            h1T0 = big.tile([P, KO, T], BF16)
            q0_ab = [big.tile([P, QCH], BF16, name=f"q0{i}") for i in range(2)]
            k0_ab = [big.tile([P, T], BF16, name=f"k0{i}") for i in range(2)]
            v0aug = big.tile([P, NT, H_CORE, HD + 1], BF16)
            yT0 = big.tile([P, 2, QCH], BF16)

            pt_pool = attn_ctx.enter_context(tc.tile_pool(name="pt", bufs=4))
            ps_ss = attn_ctx.enter_context(
                tc.tile_pool(name="ps_ss", bufs=2, space="PSUM")
            )
            ps_y = attn_ctx.enter_context(
                tc.tile_pool(name="ps_y", bufs=1, space="PSUM")
            )
            ps_mm = attn_ctx.enter_context(
                tc.tile_pool(name="ps_mm", bufs=2, space="PSUM")
            )

            nc.vector.memset(v0aug[:, :, :, HD:HD + 1], 1.0)

            for qc in range(NQC):
                c0 = True
                qcols = slice(qc * QCH, (qc + 1) * QCH)
                # ---- LN1 + h1T for this chunk's token tiles
                with tc.tile_pool(name=f"ps_tr1_{qc}", bufs=1,
                                  space="PSUM") as ps_tr1:
                    for tl in range(KPQ):
                        tt = qc * KPQ + tl
                        xt = workb.tile([P, C], BF16, tag="x_in")
                        nc.sync.dma_start(xt[:], x_bf[tt * P:(tt + 1) * P, :])
                        h_bf = ln_tile(xt, ln1_g_bc, ln1_b_bc)
                        transpose_bf(h1T0, h_bf, tt, ps_tr1)

                # ---- q/k projections
                if c0:
                    # plain bf16, head-pair blocks of 128 dims
                    for blk in range(4):
                        qk, pr = blk // 2, blk % 2
                        ps = ps_mm.tile([P, QCH], F32, tag="ps")
                        for ko in range(KO):
                            nc.tensor.matmul(
                                ps[:],
                                w_qk0_sb[:, ko, blk * P:(blk + 1) * P],
                                h1T0[:, ko, qcols],
                                start=(ko == 0), stop=(ko == KO - 1),
                            )
                        dst = (q0_ab[pr][:] if qk == 0
                               else k0_ab[pr][:, qcols])
                        if b_qk0_col is not None:
                            nc.vector.tensor_scalar_add(
                                dst, ps[:], b_qk0_col[:, blk:blk + 1]
                            )
                        else:
                            nc.vector.tensor_copy(dst, ps[:])
                else:
                    for blk in range(4):
                        qk, half = blk // 2, blk % 2
                        for pr in range(2):
                            psf = ps_mm.tile([P, QCH], F32, tag="ps")
                            ps = psf[0:64, :]
                            for k2 in range(KO // 2):
                                nc.tensor.matmul(
                                    ps,
                                    w_qk_sb[:, 2 * k2:2 * k2 + 2,
                                            (2 * blk + pr) * 64:
                                            (2 * blk + pr + 1) * 64],
                                    h1T[:, 2 * k2:2 * k2 + 2, qcols],
                                    start=(k2 == 0),
                                    stop=(k2 == KO // 2 - 1),
                                    perf_mode=DR,
                                )
                            dst = (q_ab if qk == 0 else k_ab)[pr]
                            idx = 2 * blk + pr
                            if b_qk_col is not None:
                                nc.vector.tensor_scalar_add(
                                    dst[:, half, qcols], ps,
                                    b_qk_col[:, idx:idx + 1],
                                )
                            else:
                                nc.vector.tensor_copy(dst[:, half, qcols], ps)

                # ---- v token-major (fp8-scaled path for every tile; clean
                #      bf16 path additionally for chunk-0 tiles)
                for tl in range(KPQ):
                    tt = qc * KPQ + tl
                    if True:
                        psf = ps_mm.tile([P, QCH], F32, tag="ps")
                        ps = psf[:, 0:DH]
                        for ko in range(KO):
                            nc.tensor.matmul(
                                ps[:],
                                h1T0[:, ko, tt * P:(tt + 1) * P],
                                w_v0_sb[:, ko, :],
                                start=(ko == 0), stop=(ko == KO - 1),
                            )
                        if b_v_bc is not None:
                            nc.vector.tensor_add(ps[:], ps[:], b_v_bc[:])
                        nc.vector.tensor_copy(
                            v0aug[:, tt, :, 0:HD],
                            ps[:].rearrange("p (a b) -> p a b", a=H_CORE),
                        )

                # ---- causal attention, head by head
                nkt = (qc + 1) * KPQ
                bc_ctx = tc.tile_pool(name=f"ps_bc_{qc}", bufs=1, space="PSUM")
                ps_bcp = bc_ctx.__enter__()
                escale = SCORE_SCALE0 if c0 else SCORE_SCALE
                norm_q = []
                for h in range(H_CORE):
                    hp = slice(32 * (h % 2), 32 * (h % 2) + 32)
                    hp0 = slice(64 * (h % 2), 64 * (h % 2) + 64)
                    dk, sub = h // 2, 64 * (h % 2)
                    psy = ps_y.tile([65, QCH], F32, tag="psy")
                    for ktp in range(nkt // 2):
                        kt0 = 2 * ktp
                        pss = ps_ss.tile([P, 2, QCH], F32, tag="pss")
                        pt = pt_pool.tile([P, 2, QCH], BF16, tag="pt")
                        for j in (0, 1):
                            kt = kt0 + j
                            nc.tensor.matmul(
                                pss[:, j, :],
                                k0_ab[h // 2][hp0, kt * P:(kt + 1) * P],
                                q0_ab[h // 2][hp0, :],
                                start=True, stop=True,
                            )
                        i0 = kt0 - qc * KPQ
                        for j in (0, 1):
                            i = i0 + j
                            if i >= 0:
                                if i > 0:
                                    nc.gpsimd.memset(pt[:, j, 0:i * P], 0.0)
                                nc.scalar.activation(
                                    pt[:, j, i * P:QCH],
                                    pss[:, j, i * P:QCH],
                                    mybir.ActivationFunctionType.Exp,
                                    scale=escale,
                                )
                                nc.gpsimd.tensor_mul(
                                    pt[:, j, i * P:(i + 1) * P],
                                    pt[:, j, i * P:(i + 1) * P],
                                    tri_sb[:],
                                )
                        if i0 + 1 < 0:
                            nc.scalar.activation(
                                pt[:], pss[:],
                                mybir.ActivationFunctionType.Exp,
                                scale=escale,
                            )
                        va = v0aug
                        for j in (0, 1):
                            kt = kt0 + j
                            vt = va[:, kt, h, :]
                            nc.tensor.matmul(
                                psy[:],
                                vt,
                                pt[:, j, :],
                                start=(ktp == 0 and j == 0),
                                stop=(ktp == nkt // 2 - 1 and j == 1),
                            )
                    psy_sb = rowp.tile([65, QCH], BF16, tag="psy_sb")
                    nc.scalar.copy(psy_sb[:], psy[:])
                    rrow = rowp.tile([1, QCH], mybir.dt.float32r, tag="rrow")
                    with nc.allow_low_precision(reason="softmax denom recip"):
                        nc.vector.reciprocal(rrow[:], psy_sb[64:65, :])
                    norm_q.append((psy_sb, rrow, dk, sub))
                # batched normalization: 4 bc matmuls run back-to-back on the
                # PE (no per-head DVE-dependency stall inside the score loop)
                for psy_sb, rrow, dk, sub in norm_q:
                    bc_ps = ps_bcp.tile([64, QCH], F32, tag="bc")
                    nc.tensor.matmul(
                        bc_ps[:], (ones_c0 if c0 else ones_sc)[0:1, :],
                        rrow[:], start=True, stop=True,
                    )
                    nc.vector.tensor_tensor(
                        yT0[sub:sub + 64, dk, :],
                        psy_sb[0:64, :],
                        bc_ps[:],
                        mybir.AluOpType.mult,
                    )
                bc_ctx.__exit__(None, None, None)

                # ---- out-projection partials -> RS input
                for tl in range(KPQ):
                    tt = qc * KPQ + tl
                    for nch in range(NCC):
                        ps = ps_mm.tile([P, QCH], F32, tag="ps")
                        for dk2 in range(2):
                            nc.tensor.matmul(
                                ps[:],
                                yT0[:, dk2, tl * P:(tl + 1) * P],
                                w_ap0_sb[:, dk2,
                                         nch * QCH:(nch + 1) * QCH],
                                start=(dk2 == 0), stop=(dk2 == 1),
                            )
                        ev = works.tile([P, QCH], ARDT, tag="evac")
                        nc.vector.tensor_copy(ev[:], ps[:])
                        nc.sync.dma_start(
                            rs_in[qc][tl * P:(tl + 1) * P,
                                      nch * QCH:(nch + 1) * QCH],
                            ev[:],
                        )
                if local_reduce:
                    nc.sync.dma_start(rs_out[qc][:], rs_in[qc][0:P, :])
                else:
                    nc.gpsimd.collective_compute(
                        "ReduceScatter",
                        mybir.AluOpType.add,
                        replica_groups=replica_groups,
                        ins=[rs_in[qc][:]],
                        outs=[rs_out[qc][:]],
                    )

        # ============================== MLP ===============================
        with contextlib.ExitStack() as mlp_ctx:
            x1p = mlp_ctx.enter_context(tc.tile_pool(name="x1p", bufs=1))
            x1 = x1p.tile([P, NQC, C], F32)
            w_mp_sb = x1p.tile([P, FKO, C], BF16)
            nc.sync.dma_start(w_mp_sb[:], w_mp[:])
            blkp = mlp_ctx.enter_context(tc.tile_pool(name="blkp", bufs=2))
            blkg = mlp_ctx.enter_context(tc.tile_pool(name="blkg", bufs=1))
            ps_tr2 = mlp_ctx.enter_context(
                tc.tile_pool(name="ps_tr2", bufs=2, space="PSUM")
            )
            ps_mlp = mlp_ctx.enter_context(
                tc.tile_pool(name="ps_mlp", bufs=4, space="PSUM")
            )

            for bb in range(NQC):
                # x1 = x + attn partial sum (block bb of chunk bb); the fp8
                # chunks' wire carries x(APS)-scaled partials.
                xt = workb.tile([P, C], F32, tag="x_sl")
                nc.sync.dma_start(xt[:], x_sl[bb * P:(bb + 1) * P, :])
                at = workb.tile([P, C], ARDT, tag="rs_in")
                nc.sync.dma_start(at[:], rs_out[bb][:])
                if b_ap_bc is not None:
                    nc.vector.tensor_add(xt[:], xt[:], b_ap_bc[:])
                nc.vector.scalar_tensor_tensor(
                    out=x1[:, bb, :], in0=at[:],
                    scalar=(1.0 if bb == 0 else 1.0 / APS),
                    in1=xt[:],
                    op0=mybir.AluOpType.mult, op1=mybir.AluOpType.add,
                )
                h2_bf = ln_tile(x1[:, bb, :], ln2_g_bc, ln2_b_bc)
                h2T = blkp.tile([P, KO, P], BF16, tag="h2T")
                transpose_bf(h2T, h2_bf, 0, ps_tr2)

                # fc: out [128 tok, FH] in 512-wide chunks, h2T stationary
                g_sb = blkg.tile([P, FH], BF16, tag="g_sb")
                for wave in range(2):
                    pss_fc = [
                        ps_mlp.tile([P, QCH], F32, tag="ps", name=f"fc{hc}")
                        for hc in range(4)
                    ]
                    for ko in range(KO):
                        for hc4 in range(4):
                            hc = wave * 4 + hc4
                            nc.tensor.matmul(
                                pss_fc[hc4][:],
                                h2T[:, ko, :],
                                w_fc_sb[:, ko, hc * QCH:(hc + 1) * QCH],
                                start=(ko == 0),
                                stop=(ko == KO - 1) and b_fc_row is None,
                            )
                    for hc4 in range(4):
                        hc = wave * 4 + hc4
                        if b_fc_row is not None:
                            bq = works.tile([1, QCH], BF16, tag="bq")
                            nc.vector.tensor_copy(
                                bq[:], b_fc_row[:, hc * QCH:(hc + 1) * QCH]
                            )
                            nc.tensor.matmul(
                                pss_fc[hc4][:], ones_tok[:], bq[:],
                                start=False, stop=True,
                            )
                        nc.scalar.activation(
                            g_sb[:, hc * QCH:(hc + 1) * QCH],
                            pss_fc[hc4][:],
                            mybir.ActivationFunctionType.Gelu_apprx_tanh,
                        )

                # transpose g -> gT [FH-part, tok]
                gT = blkg.tile([P, FKO, P], BF16, tag="gT")
                for kg in range(0, FKO, 4):
                    pst = ps_tr2.tile([P, 4 * P], BF16, tag="pstg")
                    for j in range(4):
                        nc.tensor.transpose(
                            pst[:, j * P:(j + 1) * P],
                            g_sb[:, (kg + j) * P:(kg + j + 1) * P],
                            ident_bf[:],
                        )
                    nc.vector.tensor_copy(
                        gT[:, kg:kg + 4, :],
                        pst[:].rearrange("p (a b) -> p a b", a=4),
                    )

                # mproj: out [128 tok, C] in 2 chunks, gT stationary
                ps_mp = [
                    ps_mlp.tile([P, QCH], F32, tag="ps", name=f"mp{n}")
                    for n in range(NCC)
                ]
                for ko in range(FKO):
                    for nch in range(NCC):
                        nc.tensor.matmul(
                            ps_mp[nch][:],
                            gT[:, ko, :],
                            w_mp_sb[:, ko, nch * QCH:(nch + 1) * QCH],
                            start=(ko == 0),
                            stop=(ko == FKO - 1) and b_mp_row is None,
                        )
                for nch in range(NCC):
                    if b_mp_row is not None:
                        bq = works.tile([1, QCH], BF16, tag="bq")
                        nc.vector.tensor_copy(
                            bq[:], b_mp_row[:, nch * QCH:(nch + 1) * QCH]
                        )
                        nc.tensor.matmul(
                            ps_mp[nch][:], ones_tok[:], bq[:],
                            start=False, stop=True,
                        )
                    ot = works.tile([P, QCH], F32, tag="out_t")
                    nc.vector.tensor_tensor(
                        ot[:], ps_mp[nch][:],
                        x1[:, bb, nch * QCH:(nch + 1) * QCH],
                        mybir.AluOpType.add,
                    )
                    nc.sync.dma_start(
                        out_y[bb * P:(bb + 1) * P, nch * QCH:(nch + 1) * QCH],
                        ot[:],
                    )

    if legalize:
        _legalize_waits(nc)
    return nc


# ---------------------------------------------------------------------------
# host-side sharding / layout prep


def _tile_k(arr, width):
    """[K, M] -> [128, K//128, M] (contraction dim inner on partitions)."""
    k, m = arr.shape
    assert m == width and k % P == 0
    return np.ascontiguousarray(
        arr.reshape(k // P, P, m).transpose(1, 0, 2)
    )


def _f8(arr):
    return np.asarray(arr, np.float32).astype(NP_FP8)


def _f8_pair(arr):
    """Return (hi, lo) fp8 decomposition of a fp32 array."""
    hi = _f8(arr)
    lo = (np.asarray(arr, np.float32) - hi.astype(np.float32)).astype(NP_FP8)
    return hi, lo


def make_core_inputs(inputs):
    f32 = np.float32
    x = np.asarray(inputs["x"], f32)
    W_attn = np.asarray(inputs["W_attn"], f32)
    W_aproj = np.asarray(inputs["W_aproj"], f32)
    W_fc = np.asarray(inputs["W_fc"], f32)
    W_mproj = np.asarray(inputs["W_mproj"], f32)
    ln1_g = np.asarray(inputs["ln1_g"], f32)
    ln1_b = np.asarray(inputs["ln1_b"], f32)
    ln2_g = np.asarray(inputs["ln2_g"], f32)
    ln2_b = np.asarray(inputs["ln2_b"], f32)
    b_attn = np.asarray(inputs["b_attn"], f32)
    b_aproj = np.asarray(inputs["b_aproj"], f32)
    b_fc = np.asarray(inputs["b_fc"], f32)
    b_mproj = np.asarray(inputs["b_mproj"], f32)

    Wq, Wk, Wv = W_attn[:C], W_attn[C:2 * C], W_attn[2 * C:]
    bq, bk, bv = b_attn[:C], b_attn[C:2 * C], b_attn[2 * C:]

    flags = set()
    if not np.all(ln1_g == 1.0):
        flags.add("ln1_g")
    if np.any(ln1_b):
        flags.add("ln1_b")
    if not np.all(ln2_g == 1.0):
        flags.add("ln2_g")
    if np.any(ln2_b):
        flags.add("ln2_b")
    if np.any(b_attn[:2 * C]):
        flags.add("b_qk")
    if np.any(bv):
        flags.add("b_v")
    if np.any(b_aproj):
        flags.add("b_ap")
    if np.any(b_fc):
        flags.add("b_fc")
    if np.any(b_mproj):
        flags.add("b_mp")

    tri = np.where(
        np.arange(P)[:, None] > np.arange(P)[None, :], f32(0.0), f32(1.0)
    ).astype(ml_dtypes.bfloat16)

    w_fc_bf = _tile_k(W_fc.T.astype(ml_dtypes.bfloat16), FH)
    w_mp_bf = _tile_k(W_mproj.T.astype(ml_dtypes.bfloat16), C)

    in_maps = []
    for core in range(N_CORES):
        g, s = core // TPG, core % TPG
        heads = list(range(s * H_CORE, (s + 1) * H_CORE))
        # fp8 path: 8 blocks of 64 rows, (qk, half, head-pair), scaled x WS
        qk_rows = []
        for W, qsc in ((Wq, 0.125), (Wk, 1.0)):
            for half in range(2):
                for pr in range(2):
                    qk_rows.append(np.concatenate(
                        [W[heads[2 * pr + hh] * HD + 32 * half:
                           heads[2 * pr + hh] * HD + 32 * half + 32]
                         * (WS * qsc) for hh in range(2)], axis=0))
        w_qk_rows = np.concatenate(qk_rows, axis=0)       # [512, C]
        # bf16 chunk-0 path: 4 blocks of 128 rows, (qk, head-pair)
        qk0_rows = []
        for W in (Wq, Wk):
            for pr in range(2):
                qk0_rows.append(np.concatenate(
                    [W[heads[2 * pr + hh] * HD:
                       heads[2 * pr + hh] * HD + HD] for hh in range(2)],
                    axis=0))
        w_qk0_rows = np.concatenate(qk0_rows, axis=0)     # [512, C]
        w_v_rows = np.concatenate(
            [Wv[h * HD:(h + 1) * HD] for h in heads], axis=0
        )                                                 # [256, C]
        dsl = slice(s * DH, (s + 1) * DH)
        tok_rows = np.concatenate(
            [x[g][qc * QCH + s * P: qc * QCH + (s + 1) * P]
             for qc in range(NQC)]
        )                                                 # [512, C]
        m = {
            "x_bf": x[g].astype(ml_dtypes.bfloat16),
            "x_sl": np.ascontiguousarray(tok_rows),
            "w_qk": _tile_k(_f8(w_qk_rows.T), 4 * P),
            "w_qk0": _tile_k(w_qk0_rows.T.astype(ml_dtypes.bfloat16), 4 * P),
            "w_v": _tile_k(_f8(w_v_rows.T * WS), DH),
            "w_v0": _tile_k(w_v_rows.T.astype(ml_dtypes.bfloat16), DH),
            "w_ap": _tile_k(_f8(W_aproj[:, dsl].T.copy() * WS), C),
            "w_ap0": _tile_k(
                W_aproj[:, dsl].T.copy().astype(ml_dtypes.bfloat16), C),
            "w_fc": w_fc_bf,
            "w_mp": w_mp_bf,
            "tri": tri,
        }
        if "ln1_g" in flags:
            m["ln1_g"] = ln1_g.reshape(1, -1).copy()
        if "ln1_b" in flags:
            m["ln1_b"] = ln1_b.reshape(1, -1).copy()
        if "ln2_g" in flags:
            m["ln2_g"] = ln2_g.reshape(1, -1).copy()
        if "ln2_b" in flags:
            m["ln2_b"] = ln2_b.reshape(1, -1).copy()
        if "b_qk" in flags:
            bqk = np.zeros((64, 8), f32)
            bi = 0
            for b_, qsc in ((bq, 0.125), (bk, 1.0)):
                for half in range(2):
                    for pr in range(2):
                        bqk[:, bi] = np.concatenate(
                            [b_[heads[2 * pr + hh] * HD + 32 * half:
                                heads[2 * pr + hh] * HD + 32 * half + 32]
                             * (WS * qsc) for hh in range(2)])
                        bi += 1
            m["b_qk"] = bqk
            bqk0 = np.zeros((P, 4), f32)
            bi = 0
            for b_ in (bq, bk):
                for pr in range(2):
                    bqk0[:, bi] = np.concatenate(
                        [b_[heads[2 * pr + hh] * HD:
                            heads[2 * pr + hh] * HD + HD]
                         for hh in range(2)])
                    bi += 1
            m["b_qk0"] = bqk0
        if "b_v" in flags:
            m["b_v"] = np.concatenate(
                [bv[h * HD:(h + 1) * HD] for h in heads]
            ).reshape(1, -1).copy()
        if "b_ap" in flags:
            m["b_ap"] = b_aproj.reshape(1, -1).copy()
        if "b_fc" in flags:
            m["b_fc"] = b_fc.reshape(1, -1).copy()
        if "b_mp" in flags:
            m["b_mp"] = b_mproj.reshape(1, -1).copy()
        in_maps.append(m)
    return in_maps, frozenset(flags)


# ---------------------------------------------------------------------------
# runner

_module_cache = {}


def run(inputs, trace=False, trace_kwargs=None, tmpdir=None):
    in_maps, flags = make_core_inputs(inputs)
    key = flags
    if key not in _module_cache:
        _module_cache[key] = build_module(flags=flags)
    nc = _module_cache[key]
    if trace:
        _install_prof_hook()
    res = run_bass_kernel_spmd(
        nc,
        in_maps,
        core_ids=list(range(N_CORES)),
        trace=trace,
        tmpdir=tmpdir,
        **(trace_kwargs or {}),
    )
    out = np.empty((B, T, C), np.float32)
    for core in range(N_CORES):
        g, s = core // TPG, core % TPG
        co = res.results[core]["out"].astype(np.float32)
        for qc in range(NQC):
            out[g, qc * QCH + s * P: qc * QCH + (s + 1) * P] = \
                co[qc * P:(qc + 1) * P]
    return out, res


def kernel(**inputs) -> np.ndarray:
    out, _ = run(inputs, trace=False)
    return out


# revision 47
# speedup vs baseline: 1.0464x; 1.0464x over previous
"""Fused causal-transformer block (LN1 -> attn -> LN2 -> MLP, residuals) on
8 Trainium2 NeuronCores — fp8 DoubleRow edition.

Sharding: tensor-parallel attention (4 heads/core) x data-parallel batch,
with a SEQUENCE-PARALLEL MLP: the attention out-projection partials are
ReduceScattered over each 4-core group, so core s owns four 128-token
blocks (one per 512-token chunk) and runs the whole MLP (full 4096 hidden)
on just those tokens.  No second collective is needed — each core writes
its own final output rows.

Compute dtype: fp8e4 (e4m3) matmul operands with DoubleRow perf mode
(2 k-tiles per instruction, 0.5 cycles/row), fp32 PSUM accumulation,
bf16 LayerNorm stream, fp32 residual stream, bf16 collective wire.

Layouts (per core):
  h1T        [128, C/128, T]   LN1 output transposed, fp8
  q_sb/k_sb  [128, 2, T]       per head h: partitions 32h..32h+31 hold the
                               two 32-dim halves of q_h (resp k_h) in free
                               slots 0/1 -> scores run as DoubleRow with
                               K=32x2 (full 64-dim dot at 0.5 cyc/row).
  vaug       [128, T/128, 4, 64] v token-major per head, fp8
  yT         [128, 2, T]       attention output transposed, fp8
  psy tile   [65, 512] PSUM    rows 0..63 = sum(p*v), row 64 = sum(p)
                               (via a ones[128,2,1] DoubleRow matmul)
"""

import contextlib
import ctypes
import math
import sys
import types

import numpy as np
import ml_dtypes

import bass_rust
import concourse.bass as bass
import concourse.mybir as mybir
import concourse.tile as tile
from concourse import library_config
from concourse.bass_utils import run_bass_kernel_spmd
from concourse.masks import make_identity
from concourse.tile import TileContext
from concourse.vector_clock import ScopedClock

# ---------------------------------------------------------------------------
# problem constants (hardcoded per the harness contract)
B, T, C, H = 2, 2048, 1024, 16
HD = C // H                 # 64
N_CORES = 8
TPG = 4                     # tensor-parallel group size
H_CORE = H // TPG           # heads per core = 4
DH = H_CORE * HD            # per-core attention dim = 256
FH = 4 * C                  # full MLP hidden (sequence-parallel MLP)
P = 128
EPS = 1e-5
QCH = 512                   # q-chunk width
NQC = T // QCH              # 4 q chunks
TSL = T // TPG              # per-core MLP token count = 512
GROUPS = [[0, 1, 2, 3], [4, 5, 6, 7]]

F32 = mybir.dt.float32
BF16 = mybir.dt.bfloat16
FP8 = mybir.dt.float8e4
NP_FP8 = ml_dtypes.float8_e4m3
DR = mybir.MatmulPerfMode.DoubleRow

# fp8 scaling: weights are quantized at x32 (fits e4m3 normal range);
# chunk-0 attention (tokens 0..511) runs in bf16 because its nearly-
# diagonal softmax gives no error averaging.  The MLP runs in bf16.
WS = 1.0                   # fp8 weight pre-scale (w_qk, w_v, w_ap)
YS = 1.0                   # y quantization pre-scale
APS = WS * YS              # aproj partial scale on the RS wire
SCORE_SCALE = 1.0          # exp scale for fp8 chunks (q pre-scaled x0.125)
SCORE_SCALE0 = 0.125       # exp scale for the bf16 chunk

# ---------------------------------------------------------------------------
# workaround 1: the container's walrus accepts a single sync-wait command per
# instruction; move extra semaphore waits onto inserted EventSemaphore
# instructions on the same engine (program order preserves semantics).

_waitfix_counter = [0]


def _legalize_waits(nc, cap=1):
    fn = nc.m.functions[0]
    n_split = 0
    for bb in fn.blocks:
        out = []
        changed = False
        for inst in bb.instructions:
            si = inst.sync_info
            waits = list(si.on_wait) if si is not None else []
            if len(waits) > cap:
                movable = [w for w in waits if w.sync_type == "semaphore"]
                fixed = [w for w in waits if w.sync_type != "semaphore"]
                n_keep = max(cap - len(fixed), 0)
                keep = fixed + (movable[len(movable) - n_keep:] if n_keep else [])
                extra = movable[: len(movable) - n_keep] if n_keep else movable
                for w in extra:
                    _waitfix_counter[0] += 1
                    ev = mybir.InstEventSemaphore(
                        name=f"I-waitfix-{_waitfix_counter[0]}",
                        engine=inst.engine,
                        ins=[],
                        outs=[],
                        sync_info=bass_rust.SyncInfo(on_wait=[w], on_update=[]),
                    )
                    out.append(ev)
                    n_split += 1
                inst.sync_info = bass_rust.SyncInfo(
                    on_wait=keep, on_update=list(si.on_update)
                )
                changed = True
            out.append(inst)
        if changed:
            bb.instructions = out
    return n_split


# workaround 2: same issue for the Tile kernel-tail Drain — emit one wait-nop
# per live proc ahead of a wait-less drain instead of stacking waits on it.


def _drain_and_barrier_split(self, tick_clock, wait_clock):
    gc = tick_clock.global_clock
    sems_alloc = wait_clock.sems.allocated()
    for proc in sorted(sems_alloc):
        tick = gc.peek_next(proc) - 1
        if tick <= 0:
            continue
        vc1 = bass_rust.VectorClock()
        vc1.require_at_least(proc, tick)
        nop = self.nc.sync.nop()
        wait_clock.add_sem_waits(nop.ins, ScopedClock({None: vc1}))
    self.nc.sync.drain()
    self.nc.all_engine_barrier()
    assert self.sems is not None
    popped = self.nc._tile_sem_poison_stack.pop()
    assert popped is self._sem_poison
    self.nc.clear_and_free_semaphores(list(self.sems.allocated().values()))
    self.nc.all_engine_barrier()


TileContext._drain_and_barrier = _drain_and_barrier_split


# workaround 3 (profiling only): register the NTFF hook the trimmed antenv
# lacks so run_bass_kernel_spmd(trace=True) works under axon.


def _install_prof_hook():
    if "antenv.axon_hooks" in sys.modules:
        return
    so_path = "/opt/axon/libaxon_pjrt.so"
    hook = None
    try:
        lib = ctypes.CDLL(so_path)
        if hasattr(lib, "axon_start_nrt_profile"):
            lib.axon_start_nrt_profile.argtypes = [
                ctypes.POINTER(ctypes.c_int64),
                ctypes.c_size_t,
            ]
            lib.axon_start_nrt_profile.restype = ctypes.c_int64
            lib.axon_stop_nrt_profile.argtypes = [ctypes.c_char_p]
            lib.axon_stop_nrt_profile.restype = ctypes.c_int64

            @contextlib.contextmanager
            def _hook_cm(output_dir, device_ids):
                import jax

                jax.devices()
                if device_ids:
                    ids = (ctypes.c_int64 * len(device_ids))(*device_ids)
                    rc = lib.axon_start_nrt_profile(ids, len(device_ids))
                else:
                    rc = lib.axon_start_nrt_profile(None, 0)
                if rc != 0:
                    raise RuntimeError(f"axon_start_nrt_profile rc={rc}")
                try:
                    yield
                finally:
                    n = lib.axon_stop_nrt_profile(str(output_dir).encode())
                    if n < 0:
                        raise RuntimeError(f"axon_stop_nrt_profile rc={n}")

            hook = _hook_cm
    except OSError:
        pass
    mod = types.ModuleType("antenv.axon_hooks")
    mod.get_axon_ntff_profile_hook = lambda: hook
    mod.set_axon_ntff_profile_hook = lambda h: None
    sys.modules["antenv.axon_hooks"] = mod
    from concourse import bass_utils

    bass_utils.upload_artifacts = lambda tmpdir: tmpdir


# ---------------------------------------------------------------------------
# device kernel builder


def build_module(
    flags=frozenset(),
    replica_groups=GROUPS,
    local_reduce=False,
    legalize=True,
):
    """Build the per-core SPMD Bass module."""
    KO = C // P                 # 8 c-tiles
    NT = T // P                 # 16 token tiles
    KPQ = QCH // P              # 4 token tiles per chunk
    FKO = FH // P               # 32 hidden tiles
    NCC = C // QCH              # 2

    nc = bass.Bass(num_devices=N_CORES)

    tri_in = nc.dram_tensor("tri", (P, P), BF16, kind="ExternalInput")
    x_bf = nc.dram_tensor("x_bf", (T, C), BF16, kind="ExternalInput")
    x_sl = nc.dram_tensor("x_sl", (TSL, C), F32, kind="ExternalInput")
    w_qk = nc.dram_tensor("w_qk", (P, KO, 4 * P), FP8, kind="ExternalInput")
    w_qk0 = nc.dram_tensor("w_qk0", (P, KO, 4 * P), BF16, kind="ExternalInput")
    w_v = nc.dram_tensor("w_v", (P, KO, DH), FP8, kind="ExternalInput")
    w_v0 = nc.dram_tensor("w_v0", (P, KO, DH), BF16, kind="ExternalInput")
    w_ap = nc.dram_tensor("w_ap", (P, 2, C), FP8, kind="ExternalInput")
    w_ap0 = nc.dram_tensor("w_ap0", (P, 2, C), BF16, kind="ExternalInput")
    w_fc = nc.dram_tensor("w_fc", (P, KO, FH), BF16, kind="ExternalInput")
    w_mp = nc.dram_tensor("w_mp", (P, FKO, C), BF16, kind="ExternalInput")
    opt_in = {}
    for name, shape in [
        ("ln1_g", (1, C)), ("ln1_b", (1, C)),
        ("ln2_g", (1, C)), ("ln2_b", (1, C)),
        ("b_qk", (64, 8)), ("b_qk0", (P, 4)), ("b_v", (1, DH)),
        ("b_ap", (1, C)), ("b_fc", (1, FH)), ("b_mp", (1, C)),
    ]:
        if name in flags or (name == "b_qk0" and "b_qk" in flags):
            opt_in[name] = nc.dram_tensor(name, shape, F32, kind="ExternalInput")

    out_y = nc.dram_tensor("out", (TSL, C), F32, kind="ExternalOutput")

    ARDT = BF16
    rs_in = [nc.dram_tensor(f"rs_in{i}", (QCH, C), ARDT) for i in range(NQC)]
    rs_out = [nc.dram_tensor(f"rs_out{i}", (P, C), ARDT) for i in range(NQC)]

    with TileContext(nc) as tc, contextlib.ExitStack() as ctx:
        const = ctx.enter_context(tc.tile_pool(name="const", bufs=1))
        workb = ctx.enter_context(tc.tile_pool(name="workb", bufs=2))
        works = ctx.enter_context(tc.tile_pool(name="works", bufs=2))
        stats = ctx.enter_context(tc.tile_pool(name="stats", bufs=6))
        rowp = ctx.enter_context(tc.tile_pool(name="rows", bufs=2))

        ident_bf = const.tile([P, P], BF16)
        make_identity(nc, ident_bf)
        ident_f8 = const.tile([P, P], FP8)
        nc.vector.tensor_copy(ident_f8[:], ident_bf[:])
        ones_c0_f = const.tile([1, 64], F32, name="ones_c0")
        nc.vector.memset(ones_c0_f[:], 1.0)
        ones_c0 = ones_c0_f[:].bitcast(mybir.dt.float32r)
        ones_sc_f = const.tile([1, 64], F32, name="ones_sc")
        nc.vector.memset(ones_sc_f[:], YS / WS)
        ones_sc = ones_sc_f[:].bitcast(mybir.dt.float32r)
        eps_t = const.tile([P, 1], F32)
        nc.vector.memset(eps_t[:], EPS)
        tri_sb = const.tile([P, P], BF16)
        nc.sync.dma_start(tri_sb[:], tri_in[:])

        def _bcast_row(name, width):
            if name not in opt_in:
                return None
            bc = const.tile([P, width], F32, name=f"bc_{name}", tag=f"bc_{name}")
            nc.sync.dma_start(bc[:], opt_in[name][:].to_broadcast((P, width)))
            return bc

        def _row(name):
            if name not in opt_in:
                return None
            t_ = const.tile(list(opt_in[name].shape), F32, name=f"r_{name}",
                            tag=f"r_{name}")
            nc.sync.dma_start(t_[:], opt_in[name][:])
            return t_

        ln1_g_bc = _bcast_row("ln1_g", C)
        ln1_b_bc = _bcast_row("ln1_b", C)
        ln2_g_bc = _bcast_row("ln2_g", C)
        ln2_b_bc = _bcast_row("ln2_b", C)
        b_v_bc = _bcast_row("b_v", DH)
        b_ap_bc = _bcast_row("b_ap", C)
        b_qk_col = _row("b_qk")      # [64, 8] for fp8 chunks
        b_qk0_col = _row("b_qk0")    # [P, 4] for the bf16 chunk
        b_fc_row = _row("b_fc")      # [1, FH]
        b_mp_row = _row("b_mp")      # [1, C]
        ones_tok = None
        if b_fc_row is not None or b_mp_row is not None:
            ones_tok = const.tile([1, P], BF16, name="ones_tok", tag="ones_tok")
            nc.vector.memset(ones_tok[:], 1.0)

        # w_fc is persistent (prefetched at kernel start; used in MLP phase)
        wfcp = ctx.enter_context(tc.tile_pool(name="wfcp", bufs=1))
        w_fc_sb = wfcp.tile([P, KO, FH], BF16)
        nc.sync.dma_start(w_fc_sb[:], w_fc[:])

        def ln_tile(x_in, g_bc, b_bc, width=C):
            """LayerNorm of a [P, width] tile -> new [P, width] bf16 tile."""
            nsub = width // 512
            st = stats.tile([P, nsub, 6], F32)
            for j in range(nsub):
                nc.vector.bn_stats(st[:, j, :], x_in[:, j * 512:(j + 1) * 512])
            mv = stats.tile([P, 2], F32)
            nc.vector.bn_aggr(mv[:], st[:])
            r = stats.tile([P, 1], F32)
            nc.scalar.activation(
                r[:], mv[:, 1:2], mybir.ActivationFunctionType.Sqrt, bias=eps_t[:]
            )
            nc.vector.reciprocal(r[:], r[:])
            h_bf = works.tile([P, width], BF16, tag="ln_out")
            if g_bc is None and b_bc is None:
                nc.vector.tensor_scalar(
                    out=h_bf[:], in0=x_in[:], scalar1=mv[:, 0:1], scalar2=r[:],
                    op0=mybir.AluOpType.subtract, op1=mybir.AluOpType.mult,
                )
            else:
                h_f = workb.tile([P, width], F32, tag="ln_f32")
                nc.vector.tensor_scalar(
                    out=h_f[:], in0=x_in[:], scalar1=mv[:, 0:1], scalar2=r[:],
                    op0=mybir.AluOpType.subtract, op1=mybir.AluOpType.mult,
                )
                if g_bc is not None:
                    nc.vector.tensor_mul(h_f[:], h_f[:], g_bc[:])
                if b_bc is not None:
                    nc.vector.tensor_add(h_f[:], h_f[:], b_bc[:])
                nc.vector.tensor_copy(h_bf[:], h_f[:])
            return h_bf

        def transpose_bf(dstT, src_bf, tt, ps_tr, nko=KO):
            """PE-transpose [P, nko*128] bf16 -> bf16 dstT[:, :, tt*P:..]."""
            for kg in range(0, nko, 4):
                nb = min(4, nko - kg)
                pst = ps_tr.tile([P, 4 * P], BF16, tag="pstb")
                for j in range(nb):
                    nc.tensor.transpose(
                        pst[:, j * P:(j + 1) * P],
                        src_bf[:, (kg + j) * P:(kg + j + 1) * P],
                        ident_bf[:],
                    )
                nc.vector.tensor_copy(
                    dstT[:, kg:kg + nb, tt * P:(tt + 1) * P],
                    pst[:, 0:nb * P].rearrange("p (a b) -> p a b", a=nb),
                )

        def transpose_f8(dstT, src_f8, tt, ps_tr, nko=KO):
            """PE-transpose [P, nko*128] fp8 -> fp8 dstT (stride-2 PSUM)."""
            for kg in range(0, nko, 4):
                nb = min(4, nko - kg)
                pst = ps_tr.tile([P, 4 * P, 2], FP8, tag="pst8")
                for j in range(nb):
                    nc.tensor.transpose(
                        pst[:, j * P:(j + 1) * P, 0:1],
                        src_f8[:, (kg + j) * P:(kg + j + 1) * P],
                        ident_f8[:],
                    )
                nc.vector.tensor_copy(
                    dstT[:, kg:kg + nb, tt * P:(tt + 1) * P],
                    pst[:, 0:nb * P, 0:1].rearrange(
                        "p (a b) o -> p a (b o)", a=nb
                    ),
                )

        # ============================ attention ============================
        with contextlib.ExitStack() as attn_ctx:
            wa = attn_ctx.enter_context(tc.tile_pool(name="wa", bufs=1))
            w_qk0_sb = wa.tile([P, KO, 4 * P], BF16)
            nc.sync.dma_start(w_qk0_sb[:], w_qk0[:])
            w_v0_sb = wa.tile([P, KO, DH], BF16)
            nc.sync.dma_start(w_v0_sb[:], w_v0[:])
            w_ap0_sb = wa.tile([P, 2, C], BF16)
            nc.sync.dma_start(w_ap0_sb[:], w_ap0[:])

            big = attn_ctx.enter_context(tc.tile_pool(name="attn_big", bufs=1))
            h1T0 = big.tile([P, KO, T], BF16)
            q0_ab = [big.tile([P, QCH], BF16, name=f"q0{i}") for i in range(2)]
            k0_ab = [big.tile([P, T], BF16, name=f"k0{i}") for i in range(2)]
            v0aug = big.tile([P, NT, H_CORE, HD + 1], BF16)
            yT0 = big.tile([P, 2, QCH], BF16)

            pt_pool = attn_ctx.enter_context(tc.tile_pool(name="pt", bufs=4))
            ps_ss = attn_ctx.enter_context(
                tc.tile_pool(name="ps_ss", bufs=2, space="PSUM")
            )
            ps_y = attn_ctx.enter_context(
                tc.tile_pool(name="ps_y", bufs=1, space="PSUM")
            )
            ps_mm = attn_ctx.enter_context(
                tc.tile_pool(name="ps_mm", bufs=2, space="PSUM")
            )

            nc.vector.memset(v0aug[:, :, :, HD:HD + 1], 1.0)

            for qc in range(NQC):
                c0 = True
                qcols = slice(qc * QCH, (qc + 1) * QCH)
                # ---- LN1 + h1T for this chunk's token tiles
                with tc.tile_pool(name=f"ps_tr1_{qc}", bufs=1,
                                  space="PSUM") as ps_tr1:
                    for tl in range(KPQ):
                        tt = qc * KPQ + tl
                        xt = workb.tile([P, C], BF16, tag="x_in")
                        nc.sync.dma_start(xt[:], x_bf[tt * P:(tt + 1) * P, :])
                        h_bf = ln_tile(xt, ln1_g_bc, ln1_b_bc)
                        transpose_bf(h1T0, h_bf, tt, ps_tr1)

                # ---- q/k projections
                if c0:
                    # plain bf16, head-pair blocks of 128 dims
                    for blk in range(4):
                        qk, pr = blk // 2, blk % 2
                        ps = ps_mm.tile([P, QCH], F32, tag="ps")
                        for ko in range(KO):
                            nc.tensor.matmul(
                                ps[:],
                                w_qk0_sb[:, ko, blk * P:(blk + 1) * P],
                                h1T0[:, ko, qcols],
                                start=(ko == 0), stop=(ko == KO - 1),
                            )
                        dst = (q0_ab[pr][:] if qk == 0
                               else k0_ab[pr][:, qcols])
                        if b_qk0_col is not None:
                            nc.vector.tensor_scalar_add(
                                dst, ps[:], b_qk0_col[:, blk:blk + 1]
                            )
                        else:
                            nc.vector.tensor_copy(dst, ps[:])
                else:
                    for blk in range(4):
                        qk, half = blk // 2, blk % 2
                        for pr in range(2):
                            psf = ps_mm.tile([P, QCH], F32, tag="ps")
                            ps = psf[0:64, :]
                            for k2 in range(KO // 2):
                                nc.tensor.matmul(
                                    ps,
                                    w_qk_sb[:, 2 * k2:2 * k2 + 2,
                                            (2 * blk + pr) * 64:
                                            (2 * blk + pr + 1) * 64],
                                    h1T[:, 2 * k2:2 * k2 + 2, qcols],
                                    start=(k2 == 0),
                                    stop=(k2 == KO // 2 - 1),
                                    perf_mode=DR,
                                )
                            dst = (q_ab if qk == 0 else k_ab)[pr]
                            idx = 2 * blk + pr
                            if b_qk_col is not None:
                                nc.vector.tensor_scalar_add(
                                    dst[:, half, qcols], ps,
                                    b_qk_col[:, idx:idx + 1],
                                )
                            else:
                                nc.vector.tensor_copy(dst[:, half, qcols], ps)

                # ---- v token-major (fp8-scaled path for every tile; clean
                #      bf16 path additionally for chunk-0 tiles)
                for tl in range(KPQ):
                    tt = qc * KPQ + tl
                    if True:
                        psf = ps_mm.tile([P, QCH], F32, tag="ps")
                        ps = psf[:, 0:DH]
                        for ko in range(KO):
                            nc.tensor.matmul(
                                ps[:],
                                h1T0[:, ko, tt * P:(tt + 1) * P],
                                w_v0_sb[:, ko, :],
                                start=(ko == 0), stop=(ko == KO - 1),
                            )
                        if b_v_bc is not None:
                            nc.vector.tensor_add(ps[:], ps[:], b_v_bc[:])
                        nc.vector.tensor_copy(
                            v0aug[:, tt, :, 0:HD],
                            ps[:].rearrange("p (a b) -> p a b", a=H_CORE),
                        )

                # ---- causal attention, head by head
                nkt = (qc + 1) * KPQ
                bc_ctx = tc.tile_pool(name=f"ps_bc_{qc}", bufs=1, space="PSUM")
                ps_bcp = bc_ctx.__enter__()
                escale = SCORE_SCALE0 if c0 else SCORE_SCALE
                for h in range(H_CORE):
                    hp = slice(32 * (h % 2), 32 * (h % 2) + 32)
                    hp0 = slice(64 * (h % 2), 64 * (h % 2) + 64)
                    dk, sub = h // 2, 64 * (h % 2)
                    psy = ps_y.tile([65, QCH], F32, tag="psy")
                    for ktp in range(nkt // 2):
                        kt0 = 2 * ktp
                        pss = ps_ss.tile([P, 2, QCH], F32, tag="pss")
                        pt = pt_pool.tile([P, 2, QCH], BF16, tag="pt")
                        for j in (0, 1):
                            kt = kt0 + j
                            nc.tensor.matmul(
                                pss[:, j, :],
                                k0_ab[h // 2][hp0, kt * P:(kt + 1) * P],
                                q0_ab[h // 2][hp0, :],
                                start=True, stop=True,
                            )
                        i0 = kt0 - qc * KPQ
                        for j in (0, 1):
                            i = i0 + j
                            if i >= 0:
                                if i > 0:
                                    nc.gpsimd.memset(pt[:, j, 0:i * P], 0.0)
                                nc.scalar.activation(
                                    pt[:, j, i * P:QCH],
                                    pss[:, j, i * P:QCH],
                                    mybir.ActivationFunctionType.Exp,
                                    scale=escale,
                                )
                                nc.gpsimd.tensor_mul(
                                    pt[:, j, i * P:(i + 1) * P],
                                    pt[:, j, i * P:(i + 1) * P],
                                    tri_sb[:],
                                )
                        if i0 + 1 < 0:
                            nc.scalar.activation(
                                pt[:], pss[:],
                                mybir.ActivationFunctionType.Exp,
                                scale=escale,
                            )
                        va = v0aug
                        for j in (0, 1):
                            kt = kt0 + j
                            vt = va[:, kt, h, :]
                            nc.tensor.matmul(
                                psy[:],
                                vt,
                                pt[:, j, :],
                                start=(ktp == 0 and j == 0),
                                stop=(ktp == nkt // 2 - 1 and j == 1),
                            )
                    # recip straight off the PSUM denom row — the bc matmul
                    # no longer waits for the full 65-row psy evacuation
                    rrow = rowp.tile([1, QCH], mybir.dt.float32r, tag="rrow")
                    with nc.allow_low_precision(reason="softmax denom recip"):
                        nc.vector.reciprocal(rrow[:], psy[64:65, :])
                    psy_sb = rowp.tile([65, QCH], BF16, tag="psy_sb")
                    nc.scalar.copy(psy_sb[:], psy[:])
                    bc_ps = ps_bcp.tile([64, QCH], F32, tag="bc")
                    nc.tensor.matmul(
                        bc_ps[:], (ones_c0 if c0 else ones_sc)[0:1, :],
                        rrow[:], start=True, stop=True,
                    )
                    nc.vector.tensor_tensor(
                        yT0[sub:sub + 64, dk, :],
                        psy_sb[0:64, :],
                        bc_ps[:],
                        mybir.AluOpType.mult,
                    )
                bc_ctx.__exit__(None, None, None)

                # ---- out-projection partials -> RS input
                for tl in range(KPQ):
                    tt = qc * KPQ + tl
                    for nch in range(NCC):
                        ps = ps_mm.tile([P, QCH], F32, tag="ps")
                        for dk2 in range(2):
                            nc.tensor.matmul(
                                ps[:],
                                yT0[:, dk2, tl * P:(tl + 1) * P],
                                w_ap0_sb[:, dk2,
                                         nch * QCH:(nch + 1) * QCH],
                                start=(dk2 == 0), stop=(dk2 == 1),
                            )
                        ev = works.tile([P, QCH], ARDT, tag="evac")
                        nc.vector.tensor_copy(ev[:], ps[:])
                        nc.sync.dma_start(
                            rs_in[qc][tl * P:(tl + 1) * P,
                                      nch * QCH:(nch + 1) * QCH],
                            ev[:],
                        )
                if local_reduce:
                    nc.sync.dma_start(rs_out[qc][:], rs_in[qc][0:P, :])
                else:
                    nc.gpsimd.collective_compute(
                        "ReduceScatter",
                        mybir.AluOpType.add,
                        replica_groups=replica_groups,
                        ins=[rs_in[qc][:]],
                        outs=[rs_out[qc][:]],
                    )

        # ============================== MLP ===============================
        with contextlib.ExitStack() as mlp_ctx:
            x1p = mlp_ctx.enter_context(tc.tile_pool(name="x1p", bufs=1))
            x1 = x1p.tile([P, NQC, C], F32)
            w_mp_sb = x1p.tile([P, FKO, C], BF16)
            nc.sync.dma_start(w_mp_sb[:], w_mp[:])
            blkp = mlp_ctx.enter_context(tc.tile_pool(name="blkp", bufs=2))
            blkg = mlp_ctx.enter_context(tc.tile_pool(name="blkg", bufs=1))
            ps_tr2 = mlp_ctx.enter_context(
                tc.tile_pool(name="ps_tr2", bufs=2, space="PSUM")
            )
            ps_mlp = mlp_ctx.enter_context(
                tc.tile_pool(name="ps_mlp", bufs=4, space="PSUM")
            )

            for bb in range(NQC):
                # x1 = x + attn partial sum (block bb of chunk bb); the fp8
                # chunks' wire carries x(APS)-scaled partials.
                xt = workb.tile([P, C], F32, tag="x_sl")
                nc.sync.dma_start(xt[:], x_sl[bb * P:(bb + 1) * P, :])
                at = workb.tile([P, C], ARDT, tag="rs_in")
                nc.sync.dma_start(at[:], rs_out[bb][:])
                if b_ap_bc is not None:
                    nc.vector.tensor_add(xt[:], xt[:], b_ap_bc[:])
                nc.vector.scalar_tensor_tensor(
                    out=x1[:, bb, :], in0=at[:],
                    scalar=(1.0 if bb == 0 else 1.0 / APS),
                    in1=xt[:],
                    op0=mybir.AluOpType.mult, op1=mybir.AluOpType.add,
                )
                h2_bf = ln_tile(x1[:, bb, :], ln2_g_bc, ln2_b_bc)
                h2T = blkp.tile([P, KO, P], BF16, tag="h2T")
                transpose_bf(h2T, h2_bf, 0, ps_tr2)

                # fc: out [128 tok, FH] in 512-wide chunks, h2T stationary
                g_sb = blkg.tile([P, FH], BF16, tag="g_sb")
                for wave in range(2):
                    pss_fc = [
                        ps_mlp.tile([P, QCH], F32, tag="ps", name=f"fc{hc}")
                        for hc in range(4)
                    ]
                    for ko in range(KO):
                        for hc4 in range(4):
                            hc = wave * 4 + hc4
                            nc.tensor.matmul(
                                pss_fc[hc4][:],
                                h2T[:, ko, :],
                                w_fc_sb[:, ko, hc * QCH:(hc + 1) * QCH],
                                start=(ko == 0),
                                stop=(ko == KO - 1) and b_fc_row is None,
                            )
                    for hc4 in range(4):
                        hc = wave * 4 + hc4
                        if b_fc_row is not None:
                            bq = works.tile([1, QCH], BF16, tag="bq")
                            nc.vector.tensor_copy(
                                bq[:], b_fc_row[:, hc * QCH:(hc + 1) * QCH]
                            )
                            nc.tensor.matmul(
                                pss_fc[hc4][:], ones_tok[:], bq[:],
                                start=False, stop=True,
                            )
                        nc.scalar.activation(
                            g_sb[:, hc * QCH:(hc + 1) * QCH],
                            pss_fc[hc4][:],
                            mybir.ActivationFunctionType.Gelu_apprx_tanh,
                        )

                # transpose g -> gT [FH-part, tok]
                gT = blkg.tile([P, FKO, P], BF16, tag="gT")
                for kg in range(0, FKO, 4):
                    pst = ps_tr2.tile([P, 4 * P], BF16, tag="pstg")
                    for j in range(4):
                        nc.tensor.transpose(
                            pst[:, j * P:(j + 1) * P],
                            g_sb[:, (kg + j) * P:(kg + j + 1) * P],
                            ident_bf[:],
                        )
                    nc.vector.tensor_copy(
                        gT[:, kg:kg + 4, :],
                        pst[:].rearrange("p (a b) -> p a b", a=4),
                    )

                # mproj: out [128 tok, C] in 2 chunks, gT stationary
                ps_mp = [
                    ps_mlp.tile([P, QCH], F32, tag="ps", name=f"mp{n}")
                    for n in range(NCC)
                ]
                for ko in range(FKO):
                    for nch in range(NCC):
                        nc.tensor.matmul(
                            ps_mp[nch][:],
                            gT[:, ko, :],
                            w_mp_sb[:, ko, nch * QCH:(nch + 1) * QCH],
                            start=(ko == 0),
                            stop=(ko == FKO - 1) and b_mp_row is None,
                        )
                for nch in range(NCC):
                    if b_mp_row is not None:
                        bq = works.tile([1, QCH], BF16, tag="bq")
                        nc.vector.tensor_copy(
                            bq[:], b_mp_row[:, nch * QCH:(nch + 1) * QCH]
                        )
                        nc.tensor.matmul(
                            ps_mp[nch][:], ones_tok[:], bq[:],
                            start=False, stop=True,
                        )
                    ot = works.tile([P, QCH], F32, tag="out_t")
                    nc.vector.tensor_tensor(
                        ot[:], ps_mp[nch][:],
                        x1[:, bb, nch * QCH:(nch + 1) * QCH],
                        mybir.AluOpType.add,
                    )
                    nc.sync.dma_start(
                        out_y[bb * P:(bb + 1) * P, nch * QCH:(nch + 1) * QCH],
                        ot[:],
                    )

    if legalize:
        _legalize_waits(nc)
    return nc


# ---------------------------------------------------------------------------
# host-side sharding / layout prep


def _tile_k(arr, width):
    """[K, M] -> [128, K//128, M] (contraction dim inner on partitions)."""
    k, m = arr.shape
    assert m == width and k % P == 0
    return np.ascontiguousarray(
        arr.reshape(k // P, P, m).transpose(1, 0, 2)
    )


def _f8(arr):
    return np.asarray(arr, np.float32).astype(NP_FP8)


def _f8_pair(arr):
    """Return (hi, lo) fp8 decomposition of a fp32 array."""
    hi = _f8(arr)
    lo = (np.asarray(arr, np.float32) - hi.astype(np.float32)).astype(NP_FP8)
    return hi, lo


def make_core_inputs(inputs):
    f32 = np.float32
    x = np.asarray(inputs["x"], f32)
    W_attn = np.asarray(inputs["W_attn"], f32)
    W_aproj = np.asarray(inputs["W_aproj"], f32)
    W_fc = np.asarray(inputs["W_fc"], f32)
    W_mproj = np.asarray(inputs["W_mproj"], f32)
    ln1_g = np.asarray(inputs["ln1_g"], f32)
    ln1_b = np.asarray(inputs["ln1_b"], f32)
    ln2_g = np.asarray(inputs["ln2_g"], f32)
    ln2_b = np.asarray(inputs["ln2_b"], f32)
    b_attn = np.asarray(inputs["b_attn"], f32)
    b_aproj = np.asarray(inputs["b_aproj"], f32)
    b_fc = np.asarray(inputs["b_fc"], f32)
    b_mproj = np.asarray(inputs["b_mproj"], f32)

    Wq, Wk, Wv = W_attn[:C], W_attn[C:2 * C], W_attn[2 * C:]
    bq, bk, bv = b_attn[:C], b_attn[C:2 * C], b_attn[2 * C:]

    flags = set()
    if not np.all(ln1_g == 1.0):
        flags.add("ln1_g")
    if np.any(ln1_b):
        flags.add("ln1_b")
    if not np.all(ln2_g == 1.0):
        flags.add("ln2_g")
    if np.any(ln2_b):
        flags.add("ln2_b")
    if np.any(b_attn[:2 * C]):
        flags.add("b_qk")
    if np.any(bv):
        flags.add("b_v")
    if np.any(b_aproj):
        flags.add("b_ap")
    if np.any(b_fc):
        flags.add("b_fc")
    if np.any(b_mproj):
        flags.add("b_mp")

    tri = np.where(
        np.arange(P)[:, None] > np.arange(P)[None, :], f32(0.0), f32(1.0)
    ).astype(ml_dtypes.bfloat16)

    w_fc_bf = _tile_k(W_fc.T.astype(ml_dtypes.bfloat16), FH)
    w_mp_bf = _tile_k(W_mproj.T.astype(ml_dtypes.bfloat16), C)

    in_maps = []
    for core in range(N_CORES):
        g, s = core // TPG, core % TPG
        heads = list(range(s * H_CORE, (s + 1) * H_CORE))
        # fp8 path: 8 blocks of 64 rows, (qk, half, head-pair), scaled x WS
        qk_rows = []
        for W, qsc in ((Wq, 0.125), (Wk, 1.0)):
            for half in range(2):
                for pr in range(2):
                    qk_rows.append(np.concatenate(
                        [W[heads[2 * pr + hh] * HD + 32 * half:
                           heads[2 * pr + hh] * HD + 32 * half + 32]
                         * (WS * qsc) for hh in range(2)], axis=0))
        w_qk_rows = np.concatenate(qk_rows, axis=0)       # [512, C]
        # bf16 chunk-0 path: 4 blocks of 128 rows, (qk, head-pair)
        qk0_rows = []
        for W in (Wq, Wk):
            for pr in range(2):
                qk0_rows.append(np.concatenate(
                    [W[heads[2 * pr + hh] * HD:
                       heads[2 * pr + hh] * HD + HD] for hh in range(2)],
                    axis=0))
        w_qk0_rows = np.concatenate(qk0_rows, axis=0)     # [512, C]
        w_v_rows = np.concatenate(
            [Wv[h * HD:(h + 1) * HD] for h in heads], axis=0
        )                                                 # [256, C]
        dsl = slice(s * DH, (s + 1) * DH)
        tok_rows = np.concatenate(
            [x[g][qc * QCH + s * P: qc * QCH + (s + 1) * P]
             for qc in range(NQC)]
        )                                                 # [512, C]
        m = {
            "x_bf": x[g].astype(ml_dtypes.bfloat16),
            "x_sl": np.ascontiguousarray(tok_rows),
            "w_qk": _tile_k(_f8(w_qk_rows.T), 4 * P),
            "w_qk0": _tile_k(w_qk0_rows.T.astype(ml_dtypes.bfloat16), 4 * P),
            "w_v": _tile_k(_f8(w_v_rows.T * WS), DH),
            "w_v0": _tile_k(w_v_rows.T.astype(ml_dtypes.bfloat16), DH),
            "w_ap": _tile_k(_f8(W_aproj[:, dsl].T.copy() * WS), C),
            "w_ap0": _tile_k(
                W_aproj[:, dsl].T.copy().astype(ml_dtypes.bfloat16), C),
            "w_fc": w_fc_bf,
            "w_mp": w_mp_bf,
            "tri": tri,
        }
        if "ln1_g" in flags:
            m["ln1_g"] = ln1_g.reshape(1, -1).copy()
        if "ln1_b" in flags:
            m["ln1_b"] = ln1_b.reshape(1, -1).copy()
        if "ln2_g" in flags:
            m["ln2_g"] = ln2_g.reshape(1, -1).copy()
        if "ln2_b" in flags:
            m["ln2_b"] = ln2_b.reshape(1, -1).copy()
        if "b_qk" in flags:
            bqk = np.zeros((64, 8), f32)
            bi = 0
            for b_, qsc in ((bq, 0.125), (bk, 1.0)):
                for half in range(2):
                    for pr in range(2):
                        bqk[:, bi] = np.concatenate(
                            [b_[heads[2 * pr + hh] * HD + 32 * half:
                                heads[2 * pr + hh] * HD + 32 * half + 32]
                             * (WS * qsc) for hh in range(2)])
                        bi += 1
            m["b_qk"] = bqk
            bqk0 = np.zeros((P, 4), f32)
            bi = 0
            for b_ in (bq, bk):
                for pr in range(2):
                    bqk0[:, bi] = np.concatenate(
                        [b_[heads[2 * pr + hh] * HD:
                            heads[2 * pr + hh] * HD + HD]
                         for hh in range(2)])
                    bi += 1
            m["b_qk0"] = bqk0
        if "b_v" in flags:
            m["b_v"] = np.concatenate(
                [bv[h * HD:(h + 1) * HD] for h in heads]
            ).reshape(1, -1).copy()
        if "b_ap" in flags:
            m["b_ap"] = b_aproj.reshape(1, -1).copy()
        if "b_fc" in flags:
            m["b_fc"] = b_fc.reshape(1, -1).copy()
        if "b_mp" in flags:
            m["b_mp"] = b_mproj.reshape(1, -1).copy()
        in_maps.append(m)
    return in_maps, frozenset(flags)


# ---------------------------------------------------------------------------
# runner

_module_cache = {}


def run(inputs, trace=False, trace_kwargs=None, tmpdir=None):
    in_maps, flags = make_core_inputs(inputs)
    key = flags
    if key not in _module_cache:
        _module_cache[key] = build_module(flags=flags)
    nc = _module_cache[key]
    if trace:
        _install_prof_hook()
    res = run_bass_kernel_spmd(
        nc,
        in_maps,
        core_ids=list(range(N_CORES)),
        trace=trace,
        tmpdir=tmpdir,
        **(trace_kwargs or {}),
    )
    out = np.empty((B, T, C), np.float32)
    for core in range(N_CORES):
        g, s = core // TPG, core % TPG
        co = res.results[core]["out"].astype(np.float32)
        for qc in range(NQC):
            out[g, qc * QCH + s * P: qc * QCH + (s + 1) * P] = \
                co[qc * P:(qc + 1) * P]
    return out, res


def kernel(**inputs) -> np.ndarray:
    out, _ = run(inputs, trace=False)
    return out
